# revision 7
# baseline (speedup 1.0000x reference)
"""2D DCT [8,32,256,256] on 8 TRN2 NeuronCores.

Math: the reference's FFT-mirror trick is exactly the linear map
    dct1d(x)[k] = (1/L) * sum_m x[m] * cos(pi*k*(m+0.5)/L)
so with A[m,k] = cos(pi*k*(m+0.5)/L)/L the 2D DCT per [256,256] slice is
    out = A^T @ X @ A = (X^T A)^T A
which is two chained TensorEngine matmuls with NO transposes:
    V  = matmul(lhsT=X, rhs=A)   # V = X^T A   (V lands [w, j] in PSUM)
    out= matmul(lhsT=V, rhs=A)   # V^T A = A^T X A  ([h', w'] in PSUM)

Sharding: fully data-parallel over the batch dim — core b takes ip[b]
(32 independent [256,256] slices). Input/output are staged as bf16 with a
host-side layout [128, 32, 2, 256] so every DMA line is contiguous per
partition; matmuls run bf16 with f32 PSUM accumulation.

Constraint in this toolchain: a lowered DMA instruction supports at most
ONE sync wait. So: all input tiles are resident (no recycle wait), each
DMA lane (8 HWDGE sems via sync, 8 SWDGE via gpsimd) is used at most
twice with the reuse carrying no data dep, and each output staging tile
has a single writer engine (one-sem wait for its DMA).
"""

import numpy as np

import concourse.bacc as bacc
import concourse.bass as bass
import concourse.mybir as mybir
import concourse.tile as tile
from concourse.bass_utils import run_bass_kernel_spmd

N_CORES = 8
C = 32                    # slices per core (channel dim; batch is sharded)
L = 256                   # DCT length
BF16 = mybir.dt.bfloat16
F32 = mybir.dt.float32
NP_BF16 = mybir.dt.np(mybir.dt.bfloat16)

# Slices per in/out DMA chunk. Small leading in-chunks start compute
# early; small trailing out-chunks shrink the drain tail. 8 chunks each
# so the 8 HWDGE lanes serve the ins and the 8 SWDGE lanes the outs.
IN_CHUNKS = [1, 1, 2, 4, 6, 6, 6, 6]
OUT_CHUNKS = [6, 6, 6, 6, 4, 2, 1, 1]


def _dct_matrix() -> np.ndarray:
    m = np.arange(L, dtype=np.float64)
    k = np.arange(L, dtype=np.float64)
    a = np.cos(np.pi * np.outer(m + 0.5, k) / L) / L
    return a.astype(np.float32).astype(NP_BF16)


def _build() -> bass.Bass:
    nc = bacc.Bacc()
    x = nc.declare_dram_parameter("x", [128, C, 2, L], BF16, isOutput=False)
    a = nc.declare_dram_parameter("dct", [L, L], BF16, isOutput=False)
    out = nc.declare_dram_parameter("out", [128, C, 2, L], BF16, isOutput=True)

    with tile.TileContext(nc) as tc:
        with (
            tc.tile_pool(name="const", bufs=1) as const_pool,
            tc.tile_pool(name="xin", bufs=1) as x_pool,
            tc.tile_pool(name="vsb", bufs=4) as v_pool,
            tc.tile_pool(name="osb", bufs=1) as o_pool,
            tc.tile_pool(name="vps", bufs=2, space="PSUM") as vps_pool,
            tc.tile_pool(name="ops", bufs=2, space="PSUM") as ops_pool,
        ):
            # A rows ki*128+p land on partition p, chunk ki.
            a_sb = const_pool.tile([128, 2, L], BF16)
            nc.sync.dma_start(a_sb[:], a.rearrange("(ki p) w -> p ki w", p=128))

            # All 32 slices stay resident (32KB/partition) — fresh tiles
            # per chunk, so in-DMAs carry no recycle wait.
            xs_tiles = {}
            c0 = 0
            for ci, n in enumerate(IN_CHUNKS):
                xt = x_pool.tile([128, n, 2, L], BF16, tag=f"x{ci}")
                nc.sync.dma_start(xt[:], x[:, c0 : c0 + n, :, :])
                for sc in range(n):
                    xs_tiles[c0 + sc] = (xt, sc)
                c0 += n

            os_tiles = {}
            os_copy_eng = {}
            c0 = 0
            for ci, n in enumerate(OUT_CHUNKS):
                ot = o_pool.tile([128, n, 2, L], BF16, tag=f"o{ci}")
                eng = nc.vector.tensor_copy if ci % 2 == 0 else nc.scalar.copy
                for sc in range(n):
                    os_tiles[c0 + sc] = (ot, sc, ci, c0 + n - 1)
                    os_copy_eng[c0 + sc] = eng
                c0 += n

            for s in range(C):
                xt, xsc = xs_tiles[s]
                ot, osc, oci, olast = os_tiles[s]
                vs = v_pool.tile([128, 2, L], BF16, tag="vs")
                vs_copy = nc.vector.tensor_copy if s % 3 else nc.scalar.copy
                for mi in range(2):
                    vp = vps_pool.tile([128, L], F32, tag=f"vp{mi}")
                    for ki in range(2):
                        nc.tensor.matmul(
                            vp[:],
                            xt[:, xsc, ki, mi * 128 : (mi + 1) * 128],
                            a_sb[:, ki, :],
                            start=(ki == 0),
                            stop=(ki == 1),
                        )
                    vs_copy(vs[:, mi, :], vp[:])
                for ji in range(2):
                    op = ops_pool.tile([128, L], F32, tag=f"op{ji}")
                    for wi in range(2):
                        nc.tensor.matmul(
                            op[:],
                            vs[:, wi, ji * 128 : (ji + 1) * 128],
                            a_sb[:, wi, :],
                            start=(wi == 0),
                            stop=(wi == 1),
                        )
                    os_copy_eng[s](ot[:, osc, ji, :], op[:])
                if s == olast:
                    # Whole out-chunk staged; SWDGE DMA (gpsimd) waits on
                    # the chunk's single copy engine only.
                    lo = s + 1 - OUT_CHUNKS[oci]
                    nc.gpsimd.dma_start(out[:, lo : s + 1, :, :], ot[:])
    nc.compile()
    return nc


_NC_CACHE: bass.Bass | None = None


def _get_nc() -> bass.Bass:
    global _NC_CACHE
    if _NC_CACHE is None:
        _NC_CACHE = _build()
    return _NC_CACHE


def _make_in_maps(ip: np.ndarray) -> list[dict[str, np.ndarray]]:
    a = _dct_matrix()
    in_maps = []
    for b in range(N_CORES):
        xb = ip[b].astype(NP_BF16)                     # [C, 256, 256]
        xb = xb.reshape(C, 2, 128, L).transpose(2, 0, 1, 3)  # [128, C, 2, L]
        in_maps.append({"x": np.ascontiguousarray(xb), "dct": a})
    return in_maps


def _unpack_out(results: list[dict[str, np.ndarray]]) -> np.ndarray:
    outs = []
    for b in range(N_CORES):
        ob = np.asarray(results[b]["out"])             # [128, C, 2, L] bf16
        ob = ob.transpose(1, 2, 0, 3).reshape(C, 256, 256).astype(np.float32)
        outs.append(ob)
    return np.stack(outs, axis=0)


def run(ip: np.ndarray, trace: bool = False):
    """Run the device kernel; returns (output, BassKernelResults)."""
    ip = np.asarray(ip)
    assert ip.shape == (N_CORES, C, 256, 256), ip.shape
    res = run_bass_kernel_spmd(
        _get_nc(), _make_in_maps(ip), core_ids=list(range(N_CORES)), trace=trace
    )
    return _unpack_out(res.results), res


def kernel(ip: np.ndarray) -> np.ndarray:
    out, _ = run(ip)
    return out


# revision 9
# speedup vs baseline: 1.1182x; 1.1182x over previous
"""2D DCT [8,32,256,256] on 8 TRN2 NeuronCores.

Math: the reference's FFT-mirror trick is exactly the linear map
    dct1d(x)[k] = (1/L) * sum_m x[m] * cos(pi*k*(m+0.5)/L)
so with A[m,k] = cos(pi*k*(m+0.5)/L)/L the 2D DCT per [256,256] slice is
    out = A^T @ X @ A = (X^T A)^T A
which is two chained TensorEngine matmuls with NO transposes:
    V  = matmul(lhsT=X, rhs=A)   # V = X^T A   (V lands [w, j] in PSUM)
    out= matmul(lhsT=V, rhs=A)   # V^T A = A^T X A  ([h', w'] in PSUM)

Sharding: fully data-parallel over the batch dim — core b takes ip[b]
(32 independent [256,256] slices). Input/output are staged as bf16 with a
host-side layout [128, 32, 2, 256] so every DMA line is contiguous per
partition; matmuls run bf16 with f32 PSUM accumulation.

Constraint in this toolchain: a lowered DMA instruction supports at most
ONE sync wait. So: all input tiles are resident (no recycle wait), each
DMA lane (8 HWDGE sems via sync, 8 SWDGE via gpsimd) is used at most
twice with the reuse carrying no data dep, and each output staging tile
has a single writer engine (one-sem wait for its DMA).
"""

import numpy as np

import concourse.bacc as bacc
import concourse.bass as bass
import concourse.mybir as mybir
import concourse.tile as tile
from concourse.bass_utils import run_bass_kernel_spmd

N_CORES = 8
C = 32                    # slices per core (channel dim; batch is sharded)
L = 256                   # DCT length
BF16 = mybir.dt.bfloat16
F32 = mybir.dt.float32
NP_BF16 = mybir.dt.np(mybir.dt.bfloat16)

# Slices per in/out DMA chunk. Small leading in-chunks start compute
# early; small trailing out-chunks shrink the drain tail. 8 chunks each
# so the 8 HWDGE lanes serve the ins and the 8 SWDGE lanes the outs.
IN_CHUNKS = [1, 1, 2, 4, 6, 6, 6, 6]
OUT_CHUNKS = [6, 6, 6, 6, 4, 2, 1, 1]


def _dct_matrix() -> np.ndarray:
    m = np.arange(L, dtype=np.float64)
    k = np.arange(L, dtype=np.float64)
    a = np.cos(np.pi * np.outer(m + 0.5, k) / L) / L
    return a.astype(np.float32).astype(NP_BF16)


def _build() -> bass.Bass:
    nc = bacc.Bacc()
    x = nc.declare_dram_parameter("x", [128, C, 2, L], BF16, isOutput=False)
    a = nc.declare_dram_parameter("dct", [L, L], BF16, isOutput=False)
    out = nc.declare_dram_parameter("out", [128, C, 2, L], BF16, isOutput=True)

    with tile.TileContext(nc) as tc:
        with (
            tc.tile_pool(name="const", bufs=1) as const_pool,
            tc.tile_pool(name="xin", bufs=1) as x_pool,
            tc.tile_pool(name="vsb", bufs=6) as v_pool,
            tc.tile_pool(name="osb", bufs=1) as o_pool,
            tc.tile_pool(name="vps", bufs=2, space="PSUM") as vps_pool,
            tc.tile_pool(name="ops", bufs=2, space="PSUM") as ops_pool,
            tc.tile_pool(name="wps", bufs=1, space="PSUM") as warm_pool,
        ):
            # HAM warm-up: ~28 dummy matmuls on garbage SBUF fill the
            # PE during the input-DMA head so real matmuls run at 2.4
            # GHz from the first slice.
            warm_sb = const_pool.tile([128, 128], BF16)
            warm_ps = warm_pool.tile([128, 128], F32)
            nc.any.memset(warm_sb[:], 0.0)
            for _ in range(28):
                nc.tensor.matmul(
                    warm_ps[:], warm_sb[:], warm_sb[:], start=True, stop=True
                )

            # A rows ki*128+p land on partition p, chunk ki. Issued on
            # the ACT HWDGE ring so it doesn't serialize with the input
            # chunks on the sync ring.
            a_sb = const_pool.tile([128, 2, L], BF16)
            nc.scalar.dma_start(a_sb[:], a.rearrange("(ki p) w -> p ki w", p=128))

            # All 32 slices stay resident (32KB/partition) — fresh tiles
            # per chunk, so in-DMAs carry no recycle wait.
            xs_tiles = {}
            c0 = 0
            for ci, n in enumerate(IN_CHUNKS):
                xt = x_pool.tile([128, n, 2, L], BF16, tag=f"x{ci}")
                nc.sync.dma_start(xt[:], x[:, c0 : c0 + n, :, :])
                for sc in range(n):
                    xs_tiles[c0 + sc] = (xt, sc)
                c0 += n

            os_tiles = {}
            c0 = 0
            for ci, n in enumerate(OUT_CHUNKS):
                ot = o_pool.tile([128, n, 2, L], BF16, tag=f"o{ci}")
                for sc in range(n):
                    os_tiles[c0 + sc] = (ot, sc, ci, c0 + n - 1)
                c0 += n

            for s in range(C):
                xt, xsc = xs_tiles[s]
                ot, osc, oci, olast = os_tiles[s]
                # One whole-bank PSUM tile and one big eviction per
                # stage; vs/os evictions split across DVE and ACT.
                vs_copy = nc.vector.tensor_copy if s % 2 else nc.scalar.copy
                os_copy = nc.scalar.copy if s % 2 else nc.vector.tensor_copy
                vs = v_pool.tile([128, 2, L], BF16, tag="vs")
                vp = vps_pool.tile([128, 2, L], F32, tag="vp")
                for mi in range(2):
                    for ki in range(2):
                        nc.tensor.matmul(
                            vp[:, mi, :],
                            xt[:, xsc, ki, mi * 128 : (mi + 1) * 128],
                            a_sb[:, ki, :],
                            start=(ki == 0),
                            stop=(ki == 1),
                        )
                vs_copy(vs[:], vp[:])
                op = ops_pool.tile([128, 2, L], F32, tag="op")
                for ji in range(2):
                    for wi in range(2):
                        nc.tensor.matmul(
                            op[:, ji, :],
                            vs[:, wi, ji * 128 : (ji + 1) * 128],
                            a_sb[:, wi, :],
                            start=(wi == 0),
                            stop=(wi == 1),
                        )
                os_copy(ot[:, osc, :, :], op[:])
                if s == olast:
                    # Whole out-chunk staged; SWDGE DMA (gpsimd) keeps
                    # the outs off the input HWDGE ring.
                    lo = s + 1 - OUT_CHUNKS[oci]
                    nc.gpsimd.dma_start(out[:, lo : s + 1, :, :], ot[:])
    nc.compile()
    return nc


_NC_CACHE: bass.Bass | None = None


def _get_nc() -> bass.Bass:
    global _NC_CACHE
    if _NC_CACHE is None:
        _NC_CACHE = _build()
    return _NC_CACHE


def _make_in_maps(ip: np.ndarray) -> list[dict[str, np.ndarray]]:
    a = _dct_matrix()
    in_maps = []
    for b in range(N_CORES):
        xb = ip[b].astype(NP_BF16)                     # [C, 256, 256]
        xb = xb.reshape(C, 2, 128, L).transpose(2, 0, 1, 3)  # [128, C, 2, L]
        in_maps.append({"x": np.ascontiguousarray(xb), "dct": a})
    return in_maps


def _unpack_out(results: list[dict[str, np.ndarray]]) -> np.ndarray:
    outs = []
    for b in range(N_CORES):
        ob = np.asarray(results[b]["out"])             # [128, C, 2, L] bf16
        ob = ob.transpose(1, 2, 0, 3).reshape(C, 256, 256).astype(np.float32)
        outs.append(ob)
    return np.stack(outs, axis=0)


def run(ip: np.ndarray, trace: bool = False):
    """Run the device kernel; returns (output, BassKernelResults)."""
    ip = np.asarray(ip)
    assert ip.shape == (N_CORES, C, 256, 256), ip.shape
    res = run_bass_kernel_spmd(
        _get_nc(), _make_in_maps(ip), core_ids=list(range(N_CORES)), trace=trace
    )
    return _unpack_out(res.results), res


def kernel(ip: np.ndarray) -> np.ndarray:
    out, _ = run(ip)
    return out
